# revision 43
# baseline (speedup 1.0000x reference)
"""Multi-head attention (b=2, n=2048, dim=1024, h=16, dh=64) on 8 TRN2 NeuronCores.

Sharding: 32 (batch, head) pairs -> 8 cores x (1 batch, 4 heads). No collectives.
Per core:
  inputs : xT  [128, 8*2048] bf16 (x[b].T packed partition-major to match the
                                   SBUF layout: element (p, kt, n) = x[b].T[kt*128+p, n])
           wq  [1024, 256]  bf16  (q-columns of w_qkv for this core's 4 heads, pre-scaled by 1/8)
           wk  [1024, 256]  bf16
           wv  [1024, 256]  bf16
  output : out [4*65, 2048] f32   (per local head: rows 0-63 = unnormalized (attn@v)^T,
                                   row 64 = softmax denominator per query)
Host divides by the denominator and transposes back to [b, n, h*dh].

Device pipeline per core:
  qT/kT = (w.T @ x.T) in [d, n] layout, head-pairs packed 2x64 on partitions (bf16)
  V     = (x @ wv)    in [n, d] layout with a ones column appended (bf16)
  per head pair, per 512-wide query chunk, per 128-wide key block:
    S^T[j,i] = kT.T @ qT   (two K=64 matmuls packed into PE row-groups 0-63 / 64-127)
    A^T      = exp(S^T)    (one ACT instr over both heads' PSUM banks, f32 -> bf16)
    O^T     += [V|1].T @ A^T  (PSUM-accumulated over key blocks; row 64 = rowsum)
"""

import numpy as np
import ml_dtypes

B, N, DIM = 2, 2048, 1024
HEADS, DH = 16, 64
P = 128
KT = DIM // P          # 8 k-tiles
NT = N // P            # 16 n/j blocks
NCH = N // 512         # 4 chunks of 512
HL = 4                 # local heads per core
OROWS = HL * DH + 8    # 256 O^T rows + 8 partial-denominator rows per core

# periods whose exp runs on the Vector engine (Schraudolph exp2 bit trick)
# instead of ACT; chosen in the ACT-bound back half of the schedule.  These
# exps are emitted as soon as their scores land (2 periods early) so the
# slower DVE path never stalls the denominator/PV consumers.
DVE_EXP = frozenset(
    [(b, jb) for b in range(3, 8) for jb in (1, 5, 9, 13)]
    + [(b, 15) for b in range(1, 8)]
    + [(7, 14)]
)
# exp(s) ~= bits_as_f32(round(2^23*(s*log2e + 127 - c))), c balances the
# mantissa-linearization error to +-3%
EXP_A = float(2.0**23 * 1.4426950408889634)
EXP_B = float(2.0**23 * (127 - 0.04303))

_CACHE = {}
LAST_RESULTS = None
TRACE = False


def _build_nc():
    from contextlib import ExitStack

    import concourse.bass as bass
    import concourse.tile as tile
    from concourse import bacc, mybir

    bf16 = mybir.dt.bfloat16
    f32 = mybir.dt.float32

    nc = bacc.Bacc("TRN2", target_bir_lowering=False)

    # xT packed n-chunk-major on host: element (p, c, kt, n) = x[b].T[kt*128+p, c*512+n]
    # so chunk c is a contiguous [128, 8KB] transfer and projections can start
    # after the first chunk lands instead of after the full 4 MiB.
    # weights pre-packed on host to [P, KT*C] (partition-major) so each DMA is
    # one contiguous 4KB/partition run (the (kt p) c layout produced 512B
    # packets and ~15us weight transfers).
    xT_d = nc.dram_tensor("xT", [P, KT * N], bf16, kind="ExternalInput")
    wq_d = nc.dram_tensor("wq", [P, KT * HL * DH], bf16, kind="ExternalInput")
    wk_d = nc.dram_tensor("wk", [P, KT * HL * DH], bf16, kind="ExternalInput")
    wv_d = nc.dram_tensor("wv", [P, KT * HL * DH], bf16, kind="ExternalInput")
    out_d = nc.dram_tensor("out", [OROWS, N], f32, kind="ExternalOutput")

    # O rows 0..255 are head-major [hl, dh, n] so a block's packed [128, 512]
    # PSUM tile (heads 2hp / 2hp+1 stacked on partitions) DMAs out as one
    # plain 2D transfer; the final 8 rows carry the per-head partial softmax
    # denominators.

    with tile.TileContext(nc) as tc, ExitStack() as ctx:
        sing = ctx.enter_context(tc.tile_pool(name="sing", bufs=1))
        spool = ctx.enter_context(
            tc.tile_pool(name="s_ps", bufs=3, space=bass.MemorySpace.PSUM)
        )
        opool = ctx.enter_context(
            tc.tile_pool(name="o_ps", bufs=1, space=bass.MemorySpace.PSUM)
        )
        apool = ctx.enter_context(tc.tile_pool(name="a_sb", bufs=14))
        copool = ctx.enter_context(tc.tile_pool(name="o_sb", bufs=4))
        ipool = ctx.enter_context(tc.tile_pool(name="i_sb", bufs=2))

        # persistent SBUF tensors; xT is [p, n-chunk, kt, 512] (chunk-major)
        xT = sing.tile([P, NCH, KT, 512], bf16, tag="xT")
        wq = sing.tile([P, KT, HL * DH], bf16, tag="wq")
        wk = sing.tile([P, KT, HL * DH], bf16, tag="wk")
        wv = sing.tile([P, KT, HL * DH], bf16, tag="wv")
        # head-pair packed projections: partitions 0-63 head A dims, 64-127 head B
        qT = [sing.tile([P, N], bf16, tag=f"qT{i}", name=f"qT{i}") for i in range(2)]
        kT = [sing.tile([P, N], bf16, tag=f"kT{i}", name=f"kT{i}") for i in range(2)]
        # V in [j, d] layout per j-block per head
        v = sing.tile([P, NT, HL, DH], bf16, tag="v")
        # ones column: stationary operand of the denominator matmuls
        ones = sing.tile([P, 1], bf16, tag="ones")

        # input DMAs, HWDGE rings only (gpsimd SWDGE has ~2us fixed costs and
        # long drains).  Scalar ring: the 3 weight tensors, k first.  Sync
        # ring: the 4 xT n-chunks in order.  Each chunk is a flat 2D
        # [128, 4096-elem] AP so the 8KB/partition contiguous run is explicit.
        # Three DMA rings in parallel (each sustains only ~140 GB/s; HBM/NC
        # allows ~358).  Critical path = wk|wq|xc0; xc1-3 split across the two
        # HWDGE rings; wq/wv ride the SWDGE (gpsimd) ring.
        xT_f = xT[:].rearrange("p c kt n -> p (c kt n)")
        nc.sync.dma_start(out=xT_f[:, 0:4096], in_=xT_d[:, 0:4096])
        nc.scalar.dma_start(out=wk[:].rearrange("p kt c -> p (kt c)"), in_=wk_d[:, :])
        nc.gpsimd.dma_start(out=wq[:].rearrange("p kt c -> p (kt c)"), in_=wq_d[:, :])
        nc.gpsimd.dma_start(out=wv[:].rearrange("p kt c -> p (kt c)"), in_=wv_d[:, :])
        for c in range(1, 4):
            h = c * 4096
            nc.sync.dma_start(out=xT_f[:, h : h + 2048], in_=xT_d[:, h : h + 2048])
            nc.scalar.dma_start(
                out=xT_f[:, h + 2048 : h + 4096], in_=xT_d[:, h + 2048 : h + 4096]
            )

        # PE warmup: the HAM clock gate keeps PE at 1.2 GHz until ~3.4us of
        # sustained activity.  Run dummy matmuls through the input-DMA wait so
        # the real projections start at the full 2.4 GHz.
        warm = sing.tile([P, 512], bf16, tag="warm")
        nc.vector.memset(warm[:], 1.0)
        wps = spool.tile([P, 512], f32, tag="sp", name="warm_ps")
        for _ in range(24):
            nc.tensor.matmul(wps[0:1, :], warm[:, 0:1], warm[:], start=True, stop=True)

        # ---- projections ----
        # k, q: out[c, n] = w[:, c].T @ xT.  hp0 upfront; hp1 woven into
        # attention-hp0's periods (PE fills slack while ACT runs exp).
        def proj_unit(wt, dst, hp, nch):
            """Emit the 8 K-accumulated matmuls + copy for one 512-col chunk,
            returned as two 4-matmul halves so weaving stays fine-grained."""
            state = {}

            def half(h):
                if h == 0:
                    state["ps"] = spool.tile([P, 512], f32, tag="sp", name="ps")
                ps = state["ps"]
                for kt in range(4 * h, 4 * h + 4):
                    nc.tensor.matmul(
                        ps[:],
                        wt[:, kt, hp * P : (hp + 1) * P],
                        xT[:, nch, kt, :],
                        start=(kt == 0),
                        stop=(kt == KT - 1),
                    )
                if h == 1:
                    nc.vector.tensor_copy(dst[:, nch * 512 : (nch + 1) * 512], ps[:])

            return [lambda: half(0), lambda: half(1)]

        nc.vector.memset(ones[:], 1.0)
        for unit in [proj_unit(wk, kT[0], 0, 0), proj_unit(wq, qT[0], 0, 0)]:
            for work in unit:
                work()

        # remaining projections are woven into the attention periods, ordered
        # by xT-chunk arrival (chunk c lands ~3us after chunk c-1); each woven
        # chunk lands (in emission order) before the first scores matmul that
        # reads it.
        def full_unit(halves):
            return lambda: [h() for h in halves]

        # V: out[n, c] = xT[ntile].T @ wv   -> [128 n, 256 c]
        def v_unit(nt):
            state = {}

            def half(h):
                if h == 0:
                    state["ps"] = spool.tile([P, HL * DH], f32, tag="sp", name="psv")
                ps = state["ps"]
                for kt in range(4 * h, 4 * h + 4):
                    nc.tensor.matmul(
                        ps[:],
                        xT[:, nt // 4, kt, (nt % 4) * P : (nt % 4 + 1) * P],
                        wv[:, kt, :],
                        start=(kt == 0),
                        stop=(kt == KT - 1),
                    )
                if h == 1:
                    nc.vector.tensor_copy(
                        v[:, nt, :, :],
                        ps[:].rearrange("p (h d) -> p h d", h=HL),
                    )

            return [lambda: half(0), lambda: half(1)]

        v_units = [full_unit(v_unit(nt)) for nt in range(NT)]
        k0 = {c: full_unit(proj_unit(wk, kT[0], 0, c)) for c in (1, 2, 3)}
        q0 = {c: full_unit(proj_unit(wq, qT[0], 0, c)) for c in (1, 2, 3)}
        # block-0 weave: kT0 chunk c must be emitted before the scores for
        # j-blocks 4c..4c+3 (scores run 2 periods ahead); v_units nt needs
        # chunk nt//4; qT0 chunk 1 before block 1's scores (emitted ~p14).
        b0_weave = {
            0: [k0[1]],
            1: [v_units[0], v_units[1]],
            2: [v_units[2], v_units[3]],
            3: [k0[2]],
            4: [v_units[4], v_units[5]],
            5: [v_units[6], v_units[7]],
            6: [k0[3]],
            7: [v_units[8], v_units[9]],
            8: [v_units[10]],
            9: [v_units[11], v_units[12]],
            10: [q0[1]],
            11: [v_units[13], v_units[14]],
            12: [v_units[15]],
            13: [q0[2]],
        }
        v_units = []  # all consumed by the block-0 weave
        # late weave, scheduled by deadline: kT1 + qT1 chunk 0 must land by
        # p62 (block 4 reads hp1); qT1 chunks 1-3 are only needed by blocks
        # 5-7, so they move into the ACT-bound back half where PE has slack.
        woven_sched = {}
        units = [q0[3]] + [full_unit(proj_unit(wk, kT[1], 1, c)) for c in range(NCH)]
        units.append(full_unit(proj_unit(wq, qT[1], 1, 0)))
        for i, u in enumerate(units):
            woven_sched[20 + 7 * i] = u
        for c, p in ((1, 70), (2, 86), (3, 102)):
            woven_sched[p] = full_unit(proj_unit(wq, qT[1], 1, c))

        # ---- attention ----
        # 8 blocks of 16 periods (one per (hp, ic)).  Exp runs one
        # [128, 1024] tile per period (ACT, or DVE via the exp2 bit trick for
        # periods in DVE_EXP); PE emits scores two periods ahead (spool
        # rotation) plus woven projection work; PV runs as dense bursts every
        # 4 periods with the two heads packed side by side in the PE array
        # (column tiles (0,0)/(0,64), both accumulating into one PSUM bank).
        # The softmax denominators come from M=1 ones-matmuls, 4 packed per
        # 512-wide span (column tiles at 0/32/64/96), accumulated over j in a
        # second PSUM bank as 2 partials per head (host adds them).
        # Block 0 weaves the V projection (PV bursts shifted late until V is
        # ready); blocks 1+ weave the remaining q/k projections.
        blocks = [(hp, ic) for hp in range(2) for ic in range(NCH)]
        ats = {}
        otiles = {}
        dtiles = {}
        sp_ahead = {}
        i32 = mybir.dt.int32

        def emit_scores(b, jb):
            hp, ic = blocks[b]
            i0, j0 = ic * 512, jb * P
            sp = spool.tile([P, 1024], f32, tag="sp", name="sp")
            nc.tensor.matmul(
                sp[:, 0:512],
                kT[hp][0:DH, j0 : j0 + P],
                qT[hp][0:DH, i0 : i0 + 512],
                start=True, stop=True, tile_position=(0, 0),
            )
            nc.tensor.matmul(
                sp[:, 512:1024],
                kT[hp][DH:P, j0 : j0 + P],
                qT[hp][DH:P, i0 : i0 + 512],
                start=True, stop=True, tile_position=(64, 0),
            )
            return sp

        def emit_exp(b, jb, sp):
            at = apool.tile([P, 1024], bf16, tag="at", name="at")
            if (b, jb) in DVE_EXP:
                it = ipool.tile([P, 1024], i32, tag="it", name="it")
                nc.vector.tensor_scalar(
                    it[:], sp[:], EXP_A, EXP_B,
                    op0=mybir.AluOpType.mult, op1=mybir.AluOpType.add,
                )
                nc.vector.tensor_copy(at[:], it[:].bitcast(f32))
            else:
                nc.scalar.activation(at[:], sp[:], mybir.ActivationFunctionType.Exp)
            ats[(b, jb)] = at

        def fetch_scores(b, jb):
            key = (b, jb)
            if key in sp_ahead:
                return sp_ahead.pop(key)
            return emit_scores(b, jb)

        def emit_d_span(b, s):
            """Denominator matmuls covering periods 2s/2s+1: 4 concurrent M=1
            column tiles (head, jb-parity), PSUM-accumulated over s."""
            if s == 0:
                dtiles[b] = opool.tile([P, 512], f32, tag="d", name="d")
            dt = dtiles[b]
            for col, h, par in ((0, 0, 0), (32, 0, 1), (64, 1, 0), (96, 1, 1)):
                nc.tensor.matmul(
                    dt[col : col + 1, :],
                    ones[:],
                    ats[(b, 2 * s + par)][:, 512 * h : 512 * h + 512],
                    start=(s == 0), stop=(s == 7),
                    tile_position=(0, col),
                )

        def emit_pv_pair(b, jb):
            """One period's PV: the two heads run concurrently as column
            tiles (0,0)/(0,64), PSUM-accumulated over jb."""
            hp, ic = blocks[b]
            if jb == 0:
                otiles[b] = opool.tile([P, 512], f32, tag="o", name="o")
            o = otiles[b]
            for col in (0, 1):
                nc.tensor.matmul(
                    o[64 * col : 64 * col + 64, :],
                    v[:, jb, 2 * hp + col, :],
                    ats[(b, jb)][:, 512 * col : 512 * col + 512],
                    start=(jb == 0), stop=(jb == NT - 1),
                    tile_position=(0, 64 * col),
                )

        def emit_out(b):
            hp, ic = blocks[b]
            i0 = ic * 512
            os = copool.tile([P, 512], f32, tag="os", name="os")
            ds = copool.tile([P, 512], f32, tag="ds", name="ds")
            if b == nblocks - 1:
                # tail: copies on two engines, the final O transfer split
                # across both DMA rings, d on sync behind the first O half
                nc.scalar.copy(os[:], otiles[b][:])
                nc.vector.tensor_copy(ds[:], dtiles[b][:])
                nc.sync.dma_start(
                    out=out_d[2 * hp * DH : (2 * hp + 1) * DH, i0 : i0 + 512],
                    in_=os[0:DH, :],
                )
                nc.scalar.dma_start(
                    out=out_d[(2 * hp + 1) * DH : (2 * hp + 2) * DH, i0 : i0 + 512],
                    in_=os[DH:P, :],
                )
                nc.sync.dma_start(
                    out=out_d[HL * DH + 4 * hp : HL * DH + 4 * hp + 4, i0 : i0 + 512],
                    in_=ds[:].rearrange("(a b) n -> a b n", b=32)[:, 0, :],
                )
            else:
                nc.vector.tensor_copy(os[:], otiles[b][:])
                nc.vector.tensor_copy(ds[:], dtiles[b][:])
                nc.sync.dma_start(
                    out=out_d[2 * hp * DH : (2 * hp + 2) * DH, i0 : i0 + 512],
                    in_=os[:],
                )
                nc.sync.dma_start(
                    out=out_d[HL * DH + 4 * hp : HL * DH + 4 * hp + 4, i0 : i0 + 512],
                    in_=ds[:].rearrange("(a b) n -> a b n", b=32)[:, 0, :],
                )

        LA = 2  # scores lookahead depth
        nblocks = len(blocks)

        def ahead(key):
            """Stage scores for `key`; DVE-offloaded periods exp immediately
            (2 periods early) so the slow path hides behind ACT's periods."""
            sp = emit_scores(*key)
            if key in DVE_EXP:
                emit_exp(*key, sp)
            else:
                sp_ahead[key] = sp

        # prime the pipeline: the first exp only waits on wk/wq + xT chunk 0
        for j in range(LA):
            ahead((0, j))
        for b in range(nblocks):
            for jb in range(NT):
                if (b, jb) not in ats:
                    emit_exp(b, jb, fetch_scores(b, jb))
                la = jb + LA
                if la < NT:
                    if (b, la) not in sp_ahead and (b, la) not in ats:
                        ahead((b, la))
                elif b + 1 < nblocks:
                    ahead((b + 1, la - NT))
                if jb == NT - 1 and b + 1 < nblocks:
                    # boundary prefetch into the idle third spool slot: gives
                    # ACT a 3rd exp of cover across the PV burst
                    ahead((b + 1, LA))
                # woven PE filler
                p = b * NT + jb
                if b == 0:
                    for work in b0_weave.get(jb, []):
                        work()
                elif p in woven_sched:
                    woven_sched[p]()
                # PV in 4-period bursts followed by the matching pair of
                # denominator spans (same at-tiles, adjacent so the PE array
                # geometry switches once, not twice); everything lags exp by
                # 1-4 periods so no exp-latency lands on PE.  Block 0's PV is
                # deferred further until the woven V projection is ready.
                if b == 0:
                    if jb in (8, 12):
                        q0_ = (jb - 8) // 4
                        for j in range(4 * q0_, 4 * q0_ + 4):
                            emit_pv_pair(0, j)
                        emit_d_span(0, 2 * q0_)
                        emit_d_span(0, 2 * q0_ + 1)
                    elif jb == NT - 1:
                        for j in range(8, NT - 1):
                            emit_pv_pair(0, j)
                        emit_d_span(0, 4)
                        emit_d_span(0, 5)
                        emit_d_span(0, 6)
                        emit_pv_pair(0, NT - 1)
                        emit_d_span(0, 7)
                        emit_out(0)
                elif jb % 4 == 0 and jb > 0:
                    for j in range(4 * (jb // 4 - 1), 4 * (jb // 4)):
                        emit_pv_pair(b, j)
                    emit_d_span(b, jb // 2 - 2)
                    emit_d_span(b, jb // 2 - 1)
                elif jb == NT - 1:
                    for j in range(NT - 4, NT - 1):
                        emit_pv_pair(b, j)
                    emit_d_span(b, 6)
                    emit_d_span(b, 7)
                    emit_pv_pair(b, NT - 1)
                    emit_out(b)

    nc.compile()
    return nc


def _get_nc():
    if "nc" not in _CACHE:
        _CACHE["nc"] = _build_nc()
    return _CACHE["nc"]


def _prepare_in_maps(x, w_qkv):
    bf = ml_dtypes.bfloat16
    x = np.asarray(x, dtype=np.float32)
    w = np.asarray(w_qkv, dtype=np.float32)
    scale = DH ** -0.5
    in_maps = []
    # n-chunk-major packing: xT[p, (c, kt, n)] = x[b].T[kt*128+p, c*512+n]
    xT_b = [
        np.ascontiguousarray(
            x[b].T.reshape(KT, P, NCH, 512).transpose(1, 2, 0, 3).reshape(P, KT * N)
        ).astype(bf)
        for b in range(B)
    ]
    def packw(wslice):
        # [1024, 256] -> [P, KT*256] partition-major (one contiguous run/partition)
        return np.ascontiguousarray(
            wslice.reshape(KT, P, HL * DH).transpose(1, 0, 2).reshape(P, KT * HL * DH)
        ).astype(bf)

    for c in range(8):
        b, hg = divmod(c, 4)
        cs = slice(hg * HL * DH, (hg + 1) * HL * DH)
        in_maps.append(
            {
                "xT": xT_b[b],
                "wq": packw(w[:, cs] * scale),
                "wk": packw(w[:, 1024:2048][:, cs]),
                "wv": packw(w[:, 2048:3072][:, cs]),
            }
        )
    return in_maps


def _assemble(outs):
    full = np.empty((B, N, HEADS * DH), dtype=np.float32)
    for c in range(8):
        b, hg = divmod(c, 4)
        o = outs[c]
        O = o[: HL * DH].reshape(HL, DH, N).copy()   # [hl, d, n] unnormalized
        dp = o[HL * DH :]                            # [8, n] partial denominators
        for h in range(HL):
            hp, col = divmod(h, 2)
            d = dp[4 * hp + 2 * col] + dp[4 * hp + 2 * col + 1]
            O[h] /= d[None, :]
        full[b, :, hg * HL * DH : (hg + 1) * HL * DH] = O.transpose(2, 0, 1).reshape(
            N, HL * DH
        )
    return full


def kernel(x, w_qkv):
    global LAST_RESULTS
    from concourse.bass_utils import run_bass_kernel_spmd

    nc = _get_nc()
    in_maps = _prepare_in_maps(x, w_qkv)
    last_err = None
    for _ in range(3):  # the runtime occasionally throws a transient device error
        try:
            res = run_bass_kernel_spmd(
                nc,
                in_maps,
                core_ids=list(range(8)),
                trace=TRACE,
                trace_cores=[0] if TRACE else None,
            )
            break
        except Exception as e:
            last_err = e
    else:
        raise last_err
    LAST_RESULTS = res
    return _assemble([r["out"] for r in res.results])



# revision 44
# speedup vs baseline: 1.0271x; 1.0271x over previous
"""Multi-head attention (b=2, n=2048, dim=1024, h=16, dh=64) on 8 TRN2 NeuronCores.

Sharding: 32 (batch, head) pairs -> 8 cores x (1 batch, 4 heads). No collectives.
Per core:
  inputs : xT  [128, 8*2048] bf16 (x[b].T packed partition-major to match the
                                   SBUF layout: element (p, kt, n) = x[b].T[kt*128+p, n])
           wq  [1024, 256]  bf16  (q-columns of w_qkv for this core's 4 heads, pre-scaled by 1/8)
           wk  [1024, 256]  bf16
           wv  [1024, 256]  bf16
  output : out [4*65, 2048] f32   (per local head: rows 0-63 = unnormalized (attn@v)^T,
                                   row 64 = softmax denominator per query)
Host divides by the denominator and transposes back to [b, n, h*dh].

Device pipeline per core:
  qT/kT = (w.T @ x.T) in [d, n] layout, head-pairs packed 2x64 on partitions (bf16)
  V     = (x @ wv)    in [n, d] layout with a ones column appended (bf16)
  per head pair, per 512-wide query chunk, per 128-wide key block:
    S^T[j,i] = kT.T @ qT   (two K=64 matmuls packed into PE row-groups 0-63 / 64-127)
    A^T      = exp(S^T)    (one ACT instr over both heads' PSUM banks, f32 -> bf16)
    O^T     += [V|1].T @ A^T  (PSUM-accumulated over key blocks; row 64 = rowsum)
"""

import numpy as np
import ml_dtypes

B, N, DIM = 2, 2048, 1024
HEADS, DH = 16, 64
P = 128
KT = DIM // P          # 8 k-tiles
NT = N // P            # 16 n/j blocks
NCH = N // 512         # 4 chunks of 512
HL = 4                 # local heads per core
OROWS = HL * DH + 8    # 256 O^T rows + 8 partial-denominator rows per core

# periods whose exp runs on the Vector engine (Schraudolph exp2 bit trick)
# instead of ACT; chosen in the ACT-bound back half of the schedule.  These
# exps are emitted as soon as their scores land (2 periods early) so the
# slower DVE path never stalls the denominator/PV consumers.
DVE_EXP = frozenset(
    [(b, jb) for b in range(3, 8) for jb in (1, 5, 9, 13)] + [(7, 14), (7, 15)]
)
# exp(s) ~= bits_as_f32(round(2^23*(s*log2e + 127 - c))), c balances the
# mantissa-linearization error to +-3%
EXP_A = float(2.0**23 * 1.4426950408889634)
EXP_B = float(2.0**23 * (127 - 0.04303))

_CACHE = {}
LAST_RESULTS = None
TRACE = False


def _build_nc():
    from contextlib import ExitStack

    import concourse.bass as bass
    import concourse.tile as tile
    from concourse import bacc, mybir

    bf16 = mybir.dt.bfloat16
    f32 = mybir.dt.float32

    nc = bacc.Bacc("TRN2", target_bir_lowering=False)

    # xT packed n-chunk-major on host: element (p, c, kt, n) = x[b].T[kt*128+p, c*512+n]
    # so chunk c is a contiguous [128, 8KB] transfer and projections can start
    # after the first chunk lands instead of after the full 4 MiB.
    # weights pre-packed on host to [P, KT*C] (partition-major) so each DMA is
    # one contiguous 4KB/partition run (the (kt p) c layout produced 512B
    # packets and ~15us weight transfers).
    xT_d = nc.dram_tensor("xT", [P, KT * N], bf16, kind="ExternalInput")
    wq_d = nc.dram_tensor("wq", [P, KT * HL * DH], bf16, kind="ExternalInput")
    wk_d = nc.dram_tensor("wk", [P, KT * HL * DH], bf16, kind="ExternalInput")
    wv_d = nc.dram_tensor("wv", [P, KT * HL * DH], bf16, kind="ExternalInput")
    out_d = nc.dram_tensor("out", [OROWS, N], f32, kind="ExternalOutput")

    # O rows 0..255 are head-major [hl, dh, n] so a block's packed [128, 512]
    # PSUM tile (heads 2hp / 2hp+1 stacked on partitions) DMAs out as one
    # plain 2D transfer; the final 8 rows carry the per-head partial softmax
    # denominators.

    with tile.TileContext(nc) as tc, ExitStack() as ctx:
        sing = ctx.enter_context(tc.tile_pool(name="sing", bufs=1))
        spool = ctx.enter_context(
            tc.tile_pool(name="s_ps", bufs=3, space=bass.MemorySpace.PSUM)
        )
        opool = ctx.enter_context(
            tc.tile_pool(name="o_ps", bufs=1, space=bass.MemorySpace.PSUM)
        )
        apool = ctx.enter_context(tc.tile_pool(name="a_sb", bufs=14))
        copool = ctx.enter_context(tc.tile_pool(name="o_sb", bufs=4))
        ipool = ctx.enter_context(tc.tile_pool(name="i_sb", bufs=2))

        # persistent SBUF tensors; xT is [p, n-chunk, kt, 512] (chunk-major)
        xT = sing.tile([P, NCH, KT, 512], bf16, tag="xT")
        wq = sing.tile([P, KT, HL * DH], bf16, tag="wq")
        wk = sing.tile([P, KT, HL * DH], bf16, tag="wk")
        wv = sing.tile([P, KT, HL * DH], bf16, tag="wv")
        # head-pair packed projections: partitions 0-63 head A dims, 64-127 head B
        qT = [sing.tile([P, N], bf16, tag=f"qT{i}", name=f"qT{i}") for i in range(2)]
        kT = [sing.tile([P, N], bf16, tag=f"kT{i}", name=f"kT{i}") for i in range(2)]
        # V in [j, d] layout per j-block per head
        v = sing.tile([P, NT, HL, DH], bf16, tag="v")
        # ones column: stationary operand of the denominator matmuls
        ones = sing.tile([P, 1], bf16, tag="ones")

        # input DMAs, HWDGE rings only (gpsimd SWDGE has ~2us fixed costs and
        # long drains).  Scalar ring: the 3 weight tensors, k first.  Sync
        # ring: the 4 xT n-chunks in order.  Each chunk is a flat 2D
        # [128, 4096-elem] AP so the 8KB/partition contiguous run is explicit.
        # Three DMA rings in parallel (each sustains only ~140 GB/s; HBM/NC
        # allows ~358).  Critical path = wk|wq|xc0; xc1-3 split across the two
        # HWDGE rings; wq/wv ride the SWDGE (gpsimd) ring.
        xT_f = xT[:].rearrange("p c kt n -> p (c kt n)")
        nc.sync.dma_start(out=xT_f[:, 0:4096], in_=xT_d[:, 0:4096])
        nc.scalar.dma_start(out=wk[:].rearrange("p kt c -> p (kt c)"), in_=wk_d[:, :])
        nc.gpsimd.dma_start(out=wq[:].rearrange("p kt c -> p (kt c)"), in_=wq_d[:, :])
        nc.gpsimd.dma_start(out=wv[:].rearrange("p kt c -> p (kt c)"), in_=wv_d[:, :])
        for c in range(1, 4):
            h = c * 4096
            nc.sync.dma_start(out=xT_f[:, h : h + 2048], in_=xT_d[:, h : h + 2048])
            nc.scalar.dma_start(
                out=xT_f[:, h + 2048 : h + 4096], in_=xT_d[:, h + 2048 : h + 4096]
            )

        # PE warmup: the HAM clock gate keeps PE at 1.2 GHz until ~3.4us of
        # sustained activity.  Run dummy matmuls through the input-DMA wait so
        # the real projections start at the full 2.4 GHz.
        warm = sing.tile([P, 512], bf16, tag="warm")
        nc.vector.memset(warm[:], 1.0)
        wps = spool.tile([P, 512], f32, tag="sp", name="warm_ps")
        for _ in range(24):
            nc.tensor.matmul(wps[0:1, :], warm[:, 0:1], warm[:], start=True, stop=True)

        # ---- projections ----
        # k, q: out[c, n] = w[:, c].T @ xT.  hp0 upfront; hp1 woven into
        # attention-hp0's periods (PE fills slack while ACT runs exp).
        def proj_unit(wt, dst, hp, nch):
            """Emit the 8 K-accumulated matmuls + copy for one 512-col chunk,
            returned as two 4-matmul halves so weaving stays fine-grained."""
            state = {}

            def half(h):
                if h == 0:
                    state["ps"] = spool.tile([P, 512], f32, tag="sp", name="ps")
                ps = state["ps"]
                for kt in range(4 * h, 4 * h + 4):
                    nc.tensor.matmul(
                        ps[:],
                        wt[:, kt, hp * P : (hp + 1) * P],
                        xT[:, nch, kt, :],
                        start=(kt == 0),
                        stop=(kt == KT - 1),
                    )
                if h == 1:
                    nc.vector.tensor_copy(dst[:, nch * 512 : (nch + 1) * 512], ps[:])

            return [lambda: half(0), lambda: half(1)]

        nc.vector.memset(ones[:], 1.0)
        for unit in [proj_unit(wk, kT[0], 0, 0), proj_unit(wq, qT[0], 0, 0)]:
            for work in unit:
                work()

        # remaining projections are woven into the attention periods, ordered
        # by xT-chunk arrival (chunk c lands ~3us after chunk c-1); each woven
        # chunk lands (in emission order) before the first scores matmul that
        # reads it.
        def full_unit(halves):
            return lambda: [h() for h in halves]

        # V: out[n, c] = xT[ntile].T @ wv   -> [128 n, 256 c]
        def v_unit(nt):
            state = {}

            def half(h):
                if h == 0:
                    state["ps"] = spool.tile([P, HL * DH], f32, tag="sp", name="psv")
                ps = state["ps"]
                for kt in range(4 * h, 4 * h + 4):
                    nc.tensor.matmul(
                        ps[:],
                        xT[:, nt // 4, kt, (nt % 4) * P : (nt % 4 + 1) * P],
                        wv[:, kt, :],
                        start=(kt == 0),
                        stop=(kt == KT - 1),
                    )
                if h == 1:
                    nc.vector.tensor_copy(
                        v[:, nt, :, :],
                        ps[:].rearrange("p (h d) -> p h d", h=HL),
                    )

            return [lambda: half(0), lambda: half(1)]

        v_units = [full_unit(v_unit(nt)) for nt in range(NT)]
        k0 = {c: full_unit(proj_unit(wk, kT[0], 0, c)) for c in (1, 2, 3)}
        q0 = {c: full_unit(proj_unit(wq, qT[0], 0, c)) for c in (1, 2, 3)}
        # block-0 weave: kT0 chunk c must be emitted before the scores for
        # j-blocks 4c..4c+3 (scores run 2 periods ahead); v_units nt needs
        # chunk nt//4; qT0 chunk 1 before block 1's scores (emitted ~p14).
        b0_weave = {
            0: [k0[1]],
            1: [v_units[0], v_units[1]],
            2: [v_units[2], v_units[3]],
            3: [k0[2]],
            4: [v_units[4], v_units[5]],
            5: [v_units[6], v_units[7]],
            6: [k0[3]],
            7: [v_units[8], v_units[9]],
            8: [v_units[10]],
            9: [v_units[11], v_units[12]],
            10: [q0[1]],
            11: [v_units[13], v_units[14]],
            12: [v_units[15]],
            13: [q0[2]],
        }
        v_units = []  # all consumed by the block-0 weave
        # late weave, scheduled by deadline: kT1 + qT1 chunk 0 must land by
        # p62 (block 4 reads hp1); qT1 chunks 1-3 are only needed by blocks
        # 5-7, so they move into the ACT-bound back half where PE has slack.
        woven_sched = {}
        units = [q0[3]] + [full_unit(proj_unit(wk, kT[1], 1, c)) for c in range(NCH)]
        units.append(full_unit(proj_unit(wq, qT[1], 1, 0)))
        for i, u in enumerate(units):
            woven_sched[20 + 7 * i] = u
        for c, p in ((1, 70), (2, 86), (3, 102)):
            woven_sched[p] = full_unit(proj_unit(wq, qT[1], 1, c))

        # ---- attention ----
        # 8 blocks of 16 periods (one per (hp, ic)).  Exp runs one
        # [128, 1024] tile per period (ACT, or DVE via the exp2 bit trick for
        # periods in DVE_EXP); PE emits scores two periods ahead (spool
        # rotation) plus woven projection work; PV runs as dense bursts every
        # 4 periods with the two heads packed side by side in the PE array
        # (column tiles (0,0)/(0,64), both accumulating into one PSUM bank).
        # The softmax denominators come from M=1 ones-matmuls, 4 packed per
        # 512-wide span (column tiles at 0/32/64/96), accumulated over j in a
        # second PSUM bank as 2 partials per head (host adds them).
        # Block 0 weaves the V projection (PV bursts shifted late until V is
        # ready); blocks 1+ weave the remaining q/k projections.
        blocks = [(hp, ic) for hp in range(2) for ic in range(NCH)]
        ats = {}
        otiles = {}
        dtiles = {}
        sp_ahead = {}
        i32 = mybir.dt.int32

        def emit_scores(b, jb):
            hp, ic = blocks[b]
            i0, j0 = ic * 512, jb * P
            sp = spool.tile([P, 1024], f32, tag="sp", name="sp")
            nc.tensor.matmul(
                sp[:, 0:512],
                kT[hp][0:DH, j0 : j0 + P],
                qT[hp][0:DH, i0 : i0 + 512],
                start=True, stop=True, tile_position=(0, 0),
            )
            nc.tensor.matmul(
                sp[:, 512:1024],
                kT[hp][DH:P, j0 : j0 + P],
                qT[hp][DH:P, i0 : i0 + 512],
                start=True, stop=True, tile_position=(64, 0),
            )
            return sp

        def emit_exp(b, jb, sp):
            at = apool.tile([P, 1024], bf16, tag="at", name="at")
            if (b, jb) in DVE_EXP:
                it = ipool.tile([P, 1024], i32, tag="it", name="it")
                nc.vector.tensor_scalar(
                    it[:], sp[:], EXP_A, EXP_B,
                    op0=mybir.AluOpType.mult, op1=mybir.AluOpType.add,
                )
                nc.vector.tensor_copy(at[:], it[:].bitcast(f32))
            else:
                nc.scalar.activation(at[:], sp[:], mybir.ActivationFunctionType.Exp)
            ats[(b, jb)] = at

        def fetch_scores(b, jb):
            key = (b, jb)
            if key in sp_ahead:
                return sp_ahead.pop(key)
            return emit_scores(b, jb)

        def emit_d_span(b, s):
            """Denominator matmuls covering periods 2s/2s+1: 4 concurrent M=1
            column tiles (head, jb-parity), PSUM-accumulated over s."""
            if s == 0:
                dtiles[b] = opool.tile([P, 512], f32, tag="d", name="d")
            dt = dtiles[b]
            for col, h, par in ((0, 0, 0), (32, 0, 1), (64, 1, 0), (96, 1, 1)):
                nc.tensor.matmul(
                    dt[col : col + 1, :],
                    ones[:],
                    ats[(b, 2 * s + par)][:, 512 * h : 512 * h + 512],
                    start=(s == 0), stop=(s == 7),
                    tile_position=(0, col),
                )

        def emit_pv_pair(b, jb):
            """One period's PV: the two heads run concurrently as column
            tiles (0,0)/(0,64), PSUM-accumulated over jb."""
            hp, ic = blocks[b]
            if jb == 0:
                otiles[b] = opool.tile([P, 512], f32, tag="o", name="o")
            o = otiles[b]
            for col in (0, 1):
                nc.tensor.matmul(
                    o[64 * col : 64 * col + 64, :],
                    v[:, jb, 2 * hp + col, :],
                    ats[(b, jb)][:, 512 * col : 512 * col + 512],
                    start=(jb == 0), stop=(jb == NT - 1),
                    tile_position=(0, 64 * col),
                )

        def emit_out(b):
            hp, ic = blocks[b]
            i0 = ic * 512
            os = copool.tile([P, 512], f32, tag="os", name="os")
            ds = copool.tile([P, 512], f32, tag="ds", name="ds")
            if b == nblocks - 1:
                # tail: copies on two engines, the final O transfer split
                # across both DMA rings, d on sync behind the first O half
                nc.scalar.copy(os[:], otiles[b][:])
                nc.vector.tensor_copy(ds[:], dtiles[b][:])
                nc.sync.dma_start(
                    out=out_d[2 * hp * DH : (2 * hp + 1) * DH, i0 : i0 + 512],
                    in_=os[0:DH, :],
                )
                nc.scalar.dma_start(
                    out=out_d[(2 * hp + 1) * DH : (2 * hp + 2) * DH, i0 : i0 + 512],
                    in_=os[DH:P, :],
                )
                nc.sync.dma_start(
                    out=out_d[HL * DH + 4 * hp : HL * DH + 4 * hp + 4, i0 : i0 + 512],
                    in_=ds[:].rearrange("(a b) n -> a b n", b=32)[:, 0, :],
                )
            else:
                nc.vector.tensor_copy(os[:], otiles[b][:])
                nc.vector.tensor_copy(ds[:], dtiles[b][:])
                nc.sync.dma_start(
                    out=out_d[2 * hp * DH : (2 * hp + 2) * DH, i0 : i0 + 512],
                    in_=os[:],
                )
                nc.sync.dma_start(
                    out=out_d[HL * DH + 4 * hp : HL * DH + 4 * hp + 4, i0 : i0 + 512],
                    in_=ds[:].rearrange("(a b) n -> a b n", b=32)[:, 0, :],
                )

        LA = 2  # scores lookahead depth
        nblocks = len(blocks)

        def ahead(key):
            """Stage scores for `key`; DVE-offloaded periods exp immediately
            (2 periods early) so the slow path hides behind ACT's periods."""
            sp = emit_scores(*key)
            if key in DVE_EXP:
                emit_exp(*key, sp)
            else:
                sp_ahead[key] = sp

        # prime the pipeline: the first exp only waits on wk/wq + xT chunk 0
        for j in range(LA):
            ahead((0, j))
        for b in range(nblocks):
            for jb in range(NT):
                if (b, jb) not in ats:
                    emit_exp(b, jb, fetch_scores(b, jb))
                la = jb + LA
                if la < NT:
                    if (b, la) not in sp_ahead and (b, la) not in ats:
                        ahead((b, la))
                elif b + 1 < nblocks:
                    ahead((b + 1, la - NT))
                if jb == NT - 1 and b + 1 < nblocks:
                    # boundary prefetch into the idle third spool slot: gives
                    # ACT a 3rd exp of cover across the PV burst
                    ahead((b + 1, LA))
                # woven PE filler
                p = b * NT + jb
                if b == 0:
                    for work in b0_weave.get(jb, []):
                        work()
                elif p in woven_sched:
                    woven_sched[p]()
                # PV in 4-period bursts followed by the matching pair of
                # denominator spans (same at-tiles, adjacent so the PE array
                # geometry switches once, not twice); everything lags exp by
                # 1-4 periods so no exp-latency lands on PE.  Block 0's PV is
                # deferred further until the woven V projection is ready.
                if b == 0:
                    if jb in (8, 12):
                        q0_ = (jb - 8) // 4
                        for j in range(4 * q0_, 4 * q0_ + 4):
                            emit_pv_pair(0, j)
                        emit_d_span(0, 2 * q0_)
                        emit_d_span(0, 2 * q0_ + 1)
                    elif jb == NT - 1:
                        for j in range(8, NT - 1):
                            emit_pv_pair(0, j)
                        emit_d_span(0, 4)
                        emit_d_span(0, 5)
                        emit_d_span(0, 6)
                        emit_pv_pair(0, NT - 1)
                        emit_d_span(0, 7)
                        emit_out(0)
                elif jb % 4 == 0 and jb > 0:
                    for j in range(4 * (jb // 4 - 1), 4 * (jb // 4)):
                        emit_pv_pair(b, j)
                    emit_d_span(b, jb // 2 - 2)
                    emit_d_span(b, jb // 2 - 1)
                elif jb == NT - 1:
                    for j in range(NT - 4, NT - 1):
                        emit_pv_pair(b, j)
                    emit_d_span(b, 6)
                    emit_d_span(b, 7)
                    emit_pv_pair(b, NT - 1)
                    emit_out(b)

    nc.compile()
    return nc


def _get_nc():
    if "nc" not in _CACHE:
        _CACHE["nc"] = _build_nc()
    return _CACHE["nc"]


def _prepare_in_maps(x, w_qkv):
    bf = ml_dtypes.bfloat16
    x = np.asarray(x, dtype=np.float32)
    w = np.asarray(w_qkv, dtype=np.float32)
    scale = DH ** -0.5
    in_maps = []
    # n-chunk-major packing: xT[p, (c, kt, n)] = x[b].T[kt*128+p, c*512+n]
    xT_b = [
        np.ascontiguousarray(
            x[b].T.reshape(KT, P, NCH, 512).transpose(1, 2, 0, 3).reshape(P, KT * N)
        ).astype(bf)
        for b in range(B)
    ]
    def packw(wslice):
        # [1024, 256] -> [P, KT*256] partition-major (one contiguous run/partition)
        return np.ascontiguousarray(
            wslice.reshape(KT, P, HL * DH).transpose(1, 0, 2).reshape(P, KT * HL * DH)
        ).astype(bf)

    for c in range(8):
        b, hg = divmod(c, 4)
        cs = slice(hg * HL * DH, (hg + 1) * HL * DH)
        in_maps.append(
            {
                "xT": xT_b[b],
                "wq": packw(w[:, cs] * scale),
                "wk": packw(w[:, 1024:2048][:, cs]),
                "wv": packw(w[:, 2048:3072][:, cs]),
            }
        )
    return in_maps


def _assemble(outs):
    full = np.empty((B, N, HEADS * DH), dtype=np.float32)
    for c in range(8):
        b, hg = divmod(c, 4)
        o = outs[c]
        O = o[: HL * DH].reshape(HL, DH, N).copy()   # [hl, d, n] unnormalized
        dp = o[HL * DH :]                            # [8, n] partial denominators
        for h in range(HL):
            hp, col = divmod(h, 2)
            d = dp[4 * hp + 2 * col] + dp[4 * hp + 2 * col + 1]
            O[h] /= d[None, :]
        full[b, :, hg * HL * DH : (hg + 1) * HL * DH] = O.transpose(2, 0, 1).reshape(
            N, HL * DH
        )
    return full


def kernel(x, w_qkv):
    global LAST_RESULTS
    from concourse.bass_utils import run_bass_kernel_spmd

    nc = _get_nc()
    in_maps = _prepare_in_maps(x, w_qkv)
    last_err = None
    for _ in range(3):  # the runtime occasionally throws a transient device error
        try:
            res = run_bass_kernel_spmd(
                nc,
                in_maps,
                core_ids=list(range(8)),
                trace=TRACE,
                trace_cores=[0] if TRACE else None,
            )
            break
        except Exception as e:
            last_err = e
    else:
        raise last_err
    LAST_RESULTS = res
    return _assemble([r["out"] for r in res.results])

